# revision 1
# baseline (speedup 1.0000x reference)
"""Trainium2 Bass kernel for nn_CPAMDec_Mix (dual cross-attention, CPAM decoder).

Math (per batch element n):
    q_i = (wq_i @ x_i + bq_i)            # (D, HW)   1x1 conv query
    k_i = y_i @ wk_i.T + bk_i            # (K, D)    linear key
    v_i = y_i @ wv_i.T + bv_i            # (K, C)    linear value
    e   = | q_1.T k_1.T - q_2.T k_2.T |  # (HW, K)
    a   = softmax_K(e)
    out_i = scale * (v_i.T @ a.T) + x_i  # (C, HW)

Sharding: pure data parallel, one batch element per NeuronCore (N=8, 8 cores).
All weights replicated.  Host-side marshaling pre-transposes the small weight
matrices / y tensors so the contraction dim (C) lands on SBUF partitions.

On-chip layout per core (everything streamed over pixel tiles of L=512):
    E^T (K x L) layout keeps softmax results directly usable as the moving
    operand of the output matmul (contract over K).  Softmax over K (the
    partition dim) is done with ones-matmuls: S = 1.T @ exp(E), then
    R = 1/S broadcast back over K partitions with another ones-matmul.
    exp() needs no max-subtraction: energies are |.| >= 0 and bounded
    (~20 for this operator scale), far from fp32 overflow.
    Matmuls run as float32r (fp32 bits, replicated fast path: 1 PE
    cycle/row for moving >= 256 instead of 4 for plain fp32).  The BIR
    verifier requires every f32r matmul operand to be produced as f32r,
    so matmul-feeding DRAM tensors/tiles are declared f32r end-to-end;
    the residual add reads the x tiles bitcast back to f32 (exact bits).
"""

import numpy as np

N, C, H, W, K = 8, 512, 64, 64, 64
HW = H * W          # 4096
D = C // 4          # 128
L = 512             # pixel tile size
NT = HW // L        # 8 tiles
NCH = C // 128      # 4 contraction chunks
P = 128

_CACHE = {}


def _build():
    from contextlib import ExitStack

    import concourse.tile as tile
    from concourse import bacc, mybir

    f32 = mybir.dt.float32
    f32r = mybir.dt.float32r
    bf16 = mybir.dt.bfloat16
    AF = mybir.ActivationFunctionType
    ALU = mybir.AluOpType

    nc = bacc.Bacc("TRN2", target_bir_lowering=False, debug=False)

    def din(name, shape, dt=f32):
        return nc.dram_tensor(name, shape, dt, kind="ExternalInput").ap()

    def dout(name, shape):
        return nc.dram_tensor(name, shape, f32, kind="ExternalOutput").ap()

    x1 = din("x1", [C, HW], f32r)
    x2 = din("x2", [C, HW], f32r)
    # k/v-side tensors come in as bf16 (they feed the bf16 E/U path)
    y1t = din("y1t", [C, K], bf16)
    y2t = din("y2t", [C, K], bf16)
    wq1t = din("wq1t", [C, D], f32r)
    wq2t = din("wq2t", [C, D], f32r)
    wk1t = din("wk1t", [C, D], bf16)
    wk2t = din("wk2t", [C, D], bf16)
    wv1t = din("wv1t", [C, C], bf16)
    wv2t = din("wv2t", [C, C], bf16)
    bq1 = din("bq1", [D, 1])
    bq2 = din("bq2", [D, 1])
    bk1 = din("bk1", [D, 1])
    bk2 = din("bk2", [D, 1])
    bv1 = din("bv1", [1, C], bf16)
    bv2 = din("bv2", [1, C], bf16)
    ones_r = din("ones_r", [1, K], bf16)
    ones_c = din("ones_c", [K, 1], bf16)
    scol = din("scol", [P, 1])  # scale broadcast to 128 partitions (host)
    o1 = dout("o1", [C, HW])
    o2 = dout("o2", [C, HW])

    # chunked (partition-major) views of the DRAM tensors
    x1r = x1.rearrange("(c p) l -> c p l", p=P)
    x2r = x2.rearrange("(c p) l -> c p l", p=P)
    o1r = o1.rearrange("(c p) l -> c p l", p=P)
    o2r = o2.rearrange("(c p) l -> c p l", p=P)
    y1r = y1t.rearrange("(c p) k -> c p k", p=P)
    y2r = y2t.rearrange("(c p) k -> c p k", p=P)
    wq1r = wq1t.rearrange("(c p) d -> c p d", p=P)
    wq2r = wq2t.rearrange("(c p) d -> c p d", p=P)
    wk1r = wk1t.rearrange("(c p) d -> c p d", p=P)
    wk2r = wk2t.rearrange("(c p) d -> c p d", p=P)
    wv1r = wv1t.rearrange("(c p) e -> c p e", p=P)
    wv2r = wv2t.rearrange("(c p) e -> c p e", p=P)

    with tile.TileContext(nc) as tc, ExitStack() as ctx:
        cpool = ctx.enter_context(tc.tile_pool(name="const", bufs=1))

        # --- load replicated constants -------------------------------------
        def load_chunks(name, src_r, nchunks, width, dt=f32r, eng=None):
            t = cpool.tile([P, nchunks * width], dt, name=name, tag=name)
            for j in range(nchunks):
                (eng or nc.sync).dma_start(
                    t[:, j * width:(j + 1) * width], src_r[j])
            return t

        # small k/q-side weights on the load (SP) ring; the big wv tensors
        # ride the otherwise-idle Activation ring so tile-0 x loads aren't
        # queued behind them
        y1s = load_chunks("y1s", y1r, NCH, K, bf16)
        y2s = load_chunks("y2s", y2r, NCH, K, bf16)
        wk1s = load_chunks("wk1s", wk1r, NCH, D, bf16)
        wk2s = load_chunks("wk2s", wk2r, NCH, D, bf16)
        wq1s = load_chunks("wq1s", wq1r, NCH, D)
        wq2s = load_chunks("wq2s", wq2r, NCH, D)
        wv1s = load_chunks("wv1s", wv1r, NCH, C, bf16, eng=nc.scalar)
        wv2s = load_chunks("wv2s", wv2r, NCH, C, bf16, eng=nc.scalar)

        def load1(name, src, shape, dt=f32):
            t = cpool.tile(shape, dt, name=name, tag=name)
            nc.sync.dma_start(t[:], src[:])
            return t

        bq1s = load1("bq1s", bq1, [D, 1])
        bq2s = load1("bq2s", bq2, [D, 1])
        bk1s = load1("bk1s", bk1, [D, 1])
        bk2s = load1("bk2s", bk2, [D, 1])
        bv1s = load1("bv1s", bv1, [1, C], bf16)
        bv2s = load1("bv2s", bv2, [1, C], bf16)
        onrs = load1("onrs", ones_r, [1, K], bf16)
        oncs = load1("oncs", ones_c, [K, 1], bf16)
        scols = load1("scols", scol, [P, 1])

        bk2n = cpool.tile([D, 1], f32, name="bk2n", tag="bk2n")
        nc.scalar.mul(bk2n[:], bk2s[:], -1.0)

        # --- setup: K1t (D,K), K2tn = -(K2t+bk2), V1 (K,C), V2 (K,C) -------
        # bf16: these feed the E/U matmuls (1 cyc/row vs 2 for f32r)
        k1s = cpool.tile([D, K], bf16, name="k1s", tag="k1s")
        k2ns = cpool.tile([D, K], bf16, name="k2ns", tag="k2ns")
        v1s = cpool.tile([K, C], bf16, name="v1s", tag="v1s")
        v2s = cpool.tile([K, C], bf16, name="v2s", tag="v2s")

        with ExitStack() as sctx:
            spsum = sctx.enter_context(
                tc.tile_pool(name="spsum", bufs=1, space="PSUM"))

            for (wks, ys, ks, bias, sc) in (
                    (wk1s, y1s, k1s, bk1s, 1.0),
                    (wk2s, y2s, k2ns, bk2n, -1.0)):
                kp = spsum.tile([D, K], f32, name="kp", tag="kp")
                for j in range(NCH):
                    nc.tensor.matmul(
                        kp[:],
                        wks[:, j * D:(j + 1) * D],
                        ys[:, j * K:(j + 1) * K],
                        start=(j == 0), stop=(j == NCH - 1))
                # ks = sc*kp + bias  (sc=-1, bias=-bk2 negates K2t + bk2)
                nc.scalar.activation(ks[:], kp[:], AF.Identity,
                                     bias=bias[:], scale=sc)

            for (ys, wvs, bvs, vs) in (
                    (y1s, wv1s, bv1s, v1s), (y2s, wv2s, bv2s, v2s)):
                vp = spsum.tile([K, C], f32, name="vp", tag="vp")
                for j in range(NCH):
                    nc.tensor.matmul(
                        vp[:],
                        ys[:, j * K:(j + 1) * K],
                        wvs[:, j * C:(j + 1) * C],
                        start=(j == 0), stop=False)
                # += ones.T @ bv  (broadcast bias add over K partitions)
                nc.tensor.matmul(vp[:], onrs[:], bvs[:], start=False,
                                 stop=True)
                nc.scalar.copy(vs[:], vp[:])

        # --- streaming pools ----------------------------------------------
        xpool = ctx.enter_context(tc.tile_pool(name="xpool", bufs=4))
        qsb = ctx.enter_context(tc.tile_pool(name="qsb", bufs=3))
        softp = ctx.enter_context(tc.tile_pool(name="softp", bufs=3))
        opool = ctx.enter_context(tc.tile_pool(name="opool", bufs=3))
        qpp = ctx.enter_context(tc.tile_pool(name="qpp", bufs=1, space="PSUM"))
        epp = ctx.enter_context(tc.tile_pool(name="epp", bufs=2, space="PSUM"))
        spp = ctx.enter_context(tc.tile_pool(name="spp", bufs=1, space="PSUM"))
        upp = ctx.enter_context(tc.tile_pool(name="upp", bufs=2, space="PSUM"))

        for t in range(NT):
            l0 = t * L
            xts = {}
            for s, xr in ((0, x1r), (1, x2r)):
                # per-stream tile holding all 4 channel chunks side by side.
                # All loads go on the SP HWDGE ring, all stores on the
                # Activation ring: a ring is FIFO, so mixing loads behind
                # compute-gated stores head-of-line-blocks the loads.
                xt = xpool.tile([P, NCH * L], f32r, name=f"x{s}", tag=f"x{s}")
                for j in range(NCH):
                    nc.sync.dma_start(xt[:, j * L:(j + 1) * L],
                                      xr[j][:, l0:l0 + L])
                xts[s] = xt

            qs = []
            for s, (wqs, bqs) in enumerate(((wq1s, bq1s), (wq2s, bq2s))):
                qp = qpp.tile([D, L], f32, name=f"q{s}p", tag=f"q{s}p")
                for j in range(NCH):
                    nc.tensor.matmul(
                        qp[:],
                        wqs[:, j * D:(j + 1) * D],
                        xts[s][:, j * L:(j + 1) * L],
                        start=(j == 0), stop=(j == NCH - 1))
                q = qsb.tile([D, L], bf16, name=f"q{s}s", tag=f"q{s}s")
                nc.scalar.activation(q[:], qp[:], AF.Identity, bias=bqs[:])
                qs.append(q)

            ep = epp.tile([K, L], f32, name="ep", tag="ep")
            nc.tensor.matmul(ep[:], k1s[:], qs[0][:], start=True, stop=False)
            nc.tensor.matmul(ep[:], k2ns[:], qs[1][:], start=False, stop=True)

            aabs = softp.tile([K, L], f32, name="aabs", tag="aabs")
            nc.scalar.activation(aabs[:], ep[:], AF.Abs)
            expe = softp.tile([K, L], bf16, name="expe", tag="expe")
            nc.scalar.activation(expe[:], aabs[:], AF.Exp)

            sp = spp.tile([1, L], f32, name="sp", tag="sp")
            nc.tensor.matmul(sp[:], oncs[:], expe[:], start=True, stop=True)
            rs = softp.tile([1, L], f32, name="rs", tag="rs")
            # 1/S at ~18 bits; S in [K, K*exp(~20)] so no edge cases
            nc.vector.reciprocal_approx_fast(rs[:], sp[:])
            rsb = softp.tile([1, L], bf16, name="rsb", tag="rsb")
            nc.scalar.copy(rsb[:], rs[:])
            rbp = spp.tile([K, L], f32, name="rbp", tag="rbp")
            nc.tensor.matmul(rbp[:], onrs[:], rsb[:], start=True, stop=True)
            attn = softp.tile([K, L], bf16, name="attn", tag="attn")
            nc.vector.tensor_mul(attn[:], expe[:], rbp[:])

            for s, (vs, orr) in enumerate(((v1s, o1r), (v2s, o2r))):
                ot = opool.tile([P, NCH * L], f32, name=f"ot{s}", tag=f"ot{s}")
                for j in range(NCH):
                    up = upp.tile([P, L], f32, name="up", tag="up")
                    nc.tensor.matmul(up[:], vs[:, j * P:(j + 1) * P],
                                     attn[:], start=True, stop=True)
                    # ot = (up * scale) + x in one DVE op
                    nc.vector.scalar_tensor_tensor(
                        ot[:, j * L:(j + 1) * L], up[:], scols[:],
                        xts[s][:, j * L:(j + 1) * L].bitcast(f32),
                        ALU.mult, ALU.add)
                    # stream-0 stores ride the SWDGE (gpsimd) queues,
                    # stream-1 the Activation HWDGE ring; the SP ring
                    # stays dedicated to loads
                    steng = nc.gpsimd if s == 0 else nc.scalar
                    steng.dma_start(orr[j][:, l0:l0 + L],
                                    ot[:, j * L:(j + 1) * L])

    nc.compile()
    return nc


def _get_nc():
    if "nc" not in _CACHE:
        try:
            import concourse  # noqa: F401
        except ImportError:
            import sys
            sys.path.insert(0, "/opt/trn_rl_repo")
        _CACHE["nc"] = _build()
    return _CACHE["nc"]


def _bf16_np():
    import ml_dtypes
    return ml_dtypes.bfloat16


def _make_in_maps(inputs):
    def f32(a):
        return np.ascontiguousarray(np.asarray(a, dtype=np.float32))

    bf = _bf16_np()

    def b16(a):
        return np.ascontiguousarray(np.asarray(a).astype(bf))

    x1 = f32(inputs["x1"]).reshape(N, C, HW)
    x2 = f32(inputs["x2"]).reshape(N, C, HW)
    y1 = np.asarray(inputs["y1"])
    y2 = np.asarray(inputs["y2"])
    shared = {
        "wq1t": f32(np.asarray(inputs["wq1"]).T),
        "wq2t": f32(np.asarray(inputs["wq2"]).T),
        "wk1t": b16(np.asarray(inputs["wk1"]).T),
        "wk2t": b16(np.asarray(inputs["wk2"]).T),
        "wv1t": b16(np.asarray(inputs["wv1"]).T),
        "wv2t": b16(np.asarray(inputs["wv2"]).T),
        "bq1": f32(inputs["bq1"]).reshape(D, 1),
        "bq2": f32(inputs["bq2"]).reshape(D, 1),
        "bk1": f32(inputs["bk1"]).reshape(D, 1),
        "bk2": f32(inputs["bk2"]).reshape(D, 1),
        "bv1": b16(np.asarray(inputs["bv1"]).reshape(1, C)),
        "bv2": b16(np.asarray(inputs["bv2"]).reshape(1, C)),
        "ones_r": np.ones((1, K), bf),
        "ones_c": np.ones((K, 1), bf),
        "scol": np.full((P, 1), np.asarray(inputs["scale"]).reshape(-1)[0],
                        dtype=np.float32),
    }
    in_maps = []
    for i in range(N):
        m = dict(shared)
        m["x1"] = x1[i]
        m["x2"] = x2[i]
        m["y1t"] = b16(y1[i].T)
        m["y2t"] = b16(y2[i].T)
        in_maps.append(m)
    return in_maps


def kernel(**inputs):
    nc = _get_nc()
    from concourse.bass_utils import run_bass_kernel_spmd

    in_maps = _make_in_maps(inputs)
    res = run_bass_kernel_spmd(nc, in_maps, list(range(N))).results
    out1 = np.stack([res[i]["o1"] for i in range(N)]).reshape(N, C, H, W)
    out2 = np.stack([res[i]["o2"] for i in range(N)]).reshape(N, C, H, W)
    return out1, out2



# revision 32
# speedup vs baseline: 1.7152x; 1.7152x over previous
"""Trainium2 Bass kernel for nn_CPAMDec_Mix (dual cross-attention, CPAM decoder).

Math (per batch element n):
    q_i = (wq_i @ x_i + bq_i)            # (D, HW)   1x1 conv query
    k_i = y_i @ wk_i.T + bk_i            # (K, D)    linear key
    v_i = y_i @ wv_i.T + bv_i            # (K, C)    linear value
    e   = | q_1.T k_1.T - q_2.T k_2.T |  # (HW, K)
    a   = softmax_K(e)
    out_i = scale * (v_i.T @ a.T) + x_i  # (C, HW)

Sharding: pure data parallel, one batch element per NeuronCore (N=8, 8 cores).
All weights replicated.

Performance design (iterated against NTFF traces; 143 us baseline -> ~84 us):
  * All device I/O is bf16 (tolerance 2e-2 >> bf16 rounding ~4e-3): x/out
    HBM traffic halves to ~16 MB/core.  The host packs x1+x2 into ONE
    [128, 32768] tensor laid out tile-major so every x load and store is a
    fully contiguous 1 MB transfer, and packs the bf16 weights into two
    const blocks (hot: y/wk/wq, cold: wv) so setup is two DMAs, not eight.
  * ALL loads ride the sync (SP HWDGE) ring in explicit priority order
    (small biases, x0, x1, wv, x2..x7): a single ring is FIFO = strict
    priority at full HBM bandwidth.  Two concurrently-active HWDGE queues
    share bandwidth weighted by packet size and starve small transfers
    (measured 269 vs 67 GB/s).  Stores also ride the sync ring - every
    load's data completes before the first store is ready, so FIFO order
    cannot starve a load.
  * Warmup bursts of dummy N=512 matmuls on memset data run during the
    initial DMA window (6 before + 5 after the k-setup), flipping the HAM
    clock gate from 1.2 to 2.4 GHz before real matmuls start: the baseline
    ran every matmul cold at (219+N)/1.2 ns.
  * Two-stage software pipeline: the softmax chain of tile t (E -> abs/exp
    -> S -> recip -> R -> attn) runs one full step ahead of tile t's output
    matmuls + drains, so the PE stream (U of t-1, E/S/R of t, q of t+1) is
    dense and the serial softmax latency is hidden.  The DVE queue is
    ordered drain,drain,recip,drain,attn,drain,drain so the critical
    reciprocal never waits behind bulk drain work.
  * Softmax over K (partition dim) via ones-matmuls; rsb = scale/S (one ACT
    row op - 2x cheaper there than DVE tensor_scalar, and catastrophic on
    GpSimd) folded so attn is pre-scaled and the residual is a plain add.
    R lands in partitions 64:128 of the E PSUM bank (saves a bank; S must
    NOT share that bank - it breaks the accumulation-group bookkeeping).
  * Residual adds run in-place into the x tile: 5 chunks/tile as direct
    PSUM->SBUF adds on DVE, 3 chunks via ACT copy + GpSimd bf16 add
    (GPSIMD cannot read PSUM).  The last tile drains entirely on DVE and
    stores per-stream so the tail is not paced by the slow GpSimd adds.
  * v-setup is emitted lazily inside step 0 so the PE never stalls on the
    late wv block; warmup/kp/vp borrow PSUM slots from the steady-state
    pools.  PSUM budget: qp(2) + ef(2) + sp(1) + up(3) = 8 banks exactly.
"""

import numpy as np

N, C, H, W, K = 8, 512, 64, 64, 64
HW = H * W          # 4096
D = C // 4          # 128
L = 512             # pixel tile size (matmul moving width)
NT = HW // L        # 8 tiles
NCH = C // 128      # 4 contraction chunks
P = 128
SWT = NCH * L       # 2048 free elems per stream within a tile block
TW = 2 * SWT        # 4096 free elems per tile block (both streams)

# packed const blocks (bf16): hot (needed first) and cold (wv, needed by U)
_CBH, _CBC = {}, {}
_off = 0
for _nm, _w in (("y1", K), ("y2", K), ("wk1", D), ("wk2", D),
                ("wq1", D), ("wq2", D)):
    _CBH[_nm] = (_off, NCH * _w)
    _off += NCH * _w
CBHW = _off         # 2560
_off = 0
for _nm, _w in (("wv1", C), ("wv2", C)):
    _CBC[_nm] = (_off, NCH * _w)
    _off += NCH * _w
CBCW = _off         # 4096

_CACHE = {}


def _build():
    from contextlib import ExitStack

    import concourse.tile as tile
    from concourse import bacc, mybir

    f32 = mybir.dt.float32
    f32r = mybir.dt.float32r
    bf16 = mybir.dt.bfloat16
    AF = mybir.ActivationFunctionType
    ALU = mybir.AluOpType

    nc = bacc.Bacc("TRN2", target_bir_lowering=False, debug=False)

    def din(name, shape, dt=bf16):
        return nc.dram_tensor(name, shape, dt, kind="ExternalInput").ap()

    xd = din("xd", [P, NT * TW])
    od = nc.dram_tensor("od", [P, NT * TW], bf16,
                        kind="ExternalOutput").ap()
    cbhd = din("cbhd", [P, CBHW])
    cbcd = din("cbcd", [P, CBCW])
    bqkd = din("bqkd", [P, 4], f32)      # cols: bq1, bq2, bk1, -bk2
    bvd = din("bvd", [1, 2 * C])         # bv1 | bv2
    scd = din("scd", [1, 1], f32)

    with tile.TileContext(nc) as tc, ExitStack() as ctx:
        cpool = ctx.enter_context(tc.tile_pool(name="const", bufs=1))

        # --- all loads on the sync ring, in priority order (ring FIFO =
        # strict priority at full HBM bandwidth; two active HWDGE queues
        # would share bandwidth weighted by packet size and starve the
        # small const transfers behind the bulk x stream).  Order: hot
        # weights, x0, small biases, x1, x2, wv block, x3..x7 ---------------
        cbhs = cpool.tile([P, CBHW], bf16, name="cbhs", tag="cbhs")
        nc.sync.dma_start(cbhs[:], cbhd[:])

        def cb(nm):
            if nm in _CBH:
                o, w = _CBH[nm]
                return cbhs[:, o:o + w]
            o, w = _CBC[nm]
            return cbcs[:, o:o + w]

        # memset-born constants: zero DMA dependency, feed the warmup burst
        onrs = cpool.tile([1, K], bf16, name="onrs", tag="onrs")
        nc.vector.memset(onrs[:], 1.0)
        oncs = cpool.tile([K, 1], bf16, name="oncs", tag="oncs")
        nc.vector.memset(oncs[:], 1.0)
        wrm = cpool.tile([K, L], bf16, name="wrm", tag="wrm")
        nc.vector.memset(wrm[:], 0.0)

        k1s = cpool.tile([D, K], bf16, name="k1s", tag="k1s")
        k2ns = cpool.tile([D, K], bf16, name="k2ns", tag="k2ns")
        v1s = cpool.tile([K, C], bf16, name="v1s", tag="v1s")
        v2s = cpool.tile([K, C], bf16, name="v2s", tag="v2s")

        # --- streaming pools (PSUM: qp 2 + ef 2 + up 2x2 = 8 banks); the
        # warmup/setup PSUM tiles borrow slots from these pools; S lands in
        # row 64 of the ef bank (safe: recip reads it before R overwrites
        # rows 64:128, enforced by the R matmul's data dependency on rs) ----
        xpool = ctx.enter_context(tc.tile_pool(name="xpool", bufs=NT))
        qsb = ctx.enter_context(tc.tile_pool(name="qsb", bufs=2))
        softp = ctx.enter_context(tc.tile_pool(name="softp", bufs=2))
        qpp = ctx.enter_context(tc.tile_pool(name="qpp", bufs=2, space="PSUM"))
        epp = ctx.enter_context(tc.tile_pool(name="epp", bufs=2, space="PSUM"))
        spp = ctx.enter_context(tc.tile_pool(name="spp", bufs=1, space="PSUM"))
        upp = ctx.enter_context(tc.tile_pool(name="upp", bufs=3, space="PSUM"))

        xts = {}

        def load_tile(t):
            xt = xpool.tile([P, TW], bf16, name=f"xd{t}", tag="xd")
            nc.sync.dma_start(xt[:], xd[:, t * TW:(t + 1) * TW])
            xts[t] = xt

        qs = {}

        def q_mm(t, s):
            wqs = cb("wq1") if s == 0 else cb("wq2")
            qp = qpp.tile([D, L], f32, name=f"qp{t}_{s}", tag="qp")
            base = s * SWT
            for j in range(NCH):
                nc.tensor.matmul(
                    qp[:],
                    wqs[:, j * D:(j + 1) * D],
                    xts[t][:, base + j * L:base + (j + 1) * L],
                    start=(j == 0), stop=(j == NCH - 1))
            return qp

        def q_act(t, s, qp):
            q = qsb.tile([D, L], bf16, name=f"q{t}_{s}", tag=f"q{s}")
            nc.scalar.activation(q[:], qp[:], AF.Identity,
                                 bias=bqks[:, s:s + 1])
            qs[(t, s)] = q

        # U-chunk helpers: chunk j of tile t -> (s, jc) = (j // 4, j % 4)
        def u_matmul(t, j):
            s, jc = divmod(j, NCH)
            vs = v1s if s == 0 else v2s
            up = upp.tile([P, L], f32, name=f"up{t}_{j}", tag="up")
            nc.tensor.matmul(up[:], vs[:, jc * P:(jc + 1) * P],
                             state[t]["attn"][:], start=True, stop=True)
            return up

        def u_dst(t, j):
            s, jc = divmod(j, NCH)
            base = s * SWT + jc * L
            return xts[t][:, base:base + L]

        def u_drain_dve(t, j, up):
            dst = u_dst(t, j)
            nc.vector.tensor_tensor(dst, up[:], dst, ALU.add)

        def u_drain_act_gps(t, j, up):
            upb = softp.tile([P, L], bf16, name=f"upb{t}_{j}",
                             tag=f"upb{j % 3}")
            nc.scalar.copy(upb[:], up[:])
            dst = u_dst(t, j)
            nc.gpsimd.tensor_tensor(dst, upb[:], dst, ALU.add)

        def store_tile(t, s=None):
            # stores ride the sync ring behind all loads: every load is
            # issued (and its data complete) before the first store is
            # ready, so FIFO order cannot starve a load, and the HWDGE
            # path keeps the expensive SWDGE emission off the GpSimd Q7
            if s is None:
                nc.sync.dma_start(od[:, t * TW:(t + 1) * TW], xts[t][:])
            else:
                nc.sync.dma_start(
                    od[:, t * TW + s * SWT:t * TW + (s + 1) * SWT],
                    xts[t][:, s * SWT:(s + 1) * SWT])

        # prologue loads in priority order behind the hot consts: tiny
        # bias/scale tensors first (they gate the k setup), then x0/x1,
        # then the wv block (first needed by U(0), one step later)
        bqks = cpool.tile([P, 4], f32, name="bqks", tag="bqks")
        nc.sync.dma_start(bqks[:], bqkd[:])
        bvs = cpool.tile([1, 2 * C], bf16, name="bvs", tag="bvs")
        nc.sync.dma_start(bvs[:], bvd[:])
        scs = cpool.tile([1, 1], f32, name="scs", tag="scs")
        nc.sync.dma_start(scs[:], scd[:])
        load_tile(0)
        load_tile(1)
        cbcs = cpool.tile([P, CBCW], bf16, name="cbcs", tag="cbcs")
        nc.sync.dma_start(cbcs[:], cbcd[:])
        for t in range(2, NT):
            load_tile(t)

        # HAM warmup: ~3.5 us of back-to-back cold matmuls while the first
        # DMAs land; flips the PE clock gate to 2.4 GHz.  Borrows the sp
        # PSUM slot (never read; later writers order behind it harmlessly).
        wp = spp.tile([1, L], f32, name="wp", tag="sp")
        for _ in range(6):
            nc.tensor.matmul(wp[:], oncs[:], wrm[:], start=True, stop=True)

        # K1t (D,K), K2tn = -(K2t + bk2); kp borrows qp slots
        for (wkn, yn, ks, bcol, sc) in (
                ("wk1", "y1", k1s, 2, 1.0),
                ("wk2", "y2", k2ns, 3, -1.0)):
            kp = qpp.tile([D, K], f32, name="kp", tag="qp")
            wks, ys = cb(wkn), cb(yn)
            for j in range(NCH):
                nc.tensor.matmul(
                    kp[:],
                    wks[:, j * D:(j + 1) * D],
                    ys[:, j * K:(j + 1) * K],
                    start=(j == 0), stop=(j == NCH - 1))
            # ks = sc*kp + bias  (sc=-1, bias=-bk2 negates K2t + bk2)
            nc.scalar.activation(ks[:], kp[:], AF.Identity,
                                 bias=bqks[:, bcol:bcol + 1], scale=sc)

        # second warmup burst: keeps the PE busy (and the clock gate warm)
        # across the wait for the x0 load
        for _ in range(5):
            nc.tensor.matmul(wp[:], oncs[:], wrm[:], start=True, stop=True)

        q_act(0, 0, q_mm(0, 0))
        q_act(0, 1, q_mm(0, 1))

        def v_setup():
            # V1, V2 (K, C): emitted lazily inside step 0 so the PE does not
            # stall on the (late-loaded) wv block before tile 0's softmax;
            # vp borrows up slots
            for si, (yn, wvn, vs) in enumerate(
                    (("y1", "wv1", v1s), ("y2", "wv2", v2s))):
                vp = upp.tile([K, C], f32, name=f"vp{si}", tag="up")
                ys, wvs = cb(yn), cb(wvn)
                for j in range(NCH):
                    nc.tensor.matmul(
                        vp[:],
                        ys[:, j * K:(j + 1) * K],
                        wvs[:, j * C:(j + 1) * C],
                        start=(j == 0), stop=False)
                nc.tensor.matmul(vp[:], onrs[:], bvs[:, si * C:(si + 1) * C],
                                 start=False, stop=True)
                nc.scalar.copy(vs[:], vp[:])

        state = {}
        for i in range(NT + 1):
            tU = i - 1          # tile in output/drain stage
            tS = i              # tile in softmax stage
            tQ = i + 1          # tile in q stage
            ups = {}

            # PE: U0,U1 of tU; DVE: their drains ahead of recip
            if tU >= 0:
                for j in (0, 1):
                    ups[j] = u_matmul(tU, j)
                    u_drain_dve(tU, j, ups[j])

            if tS < NT:
                ef = epp.tile([P, L], f32, name=f"ef{tS}", tag="ep")
                st = state[tS] = {"ef": ef}
                nc.tensor.matmul(ef[0:K, :], k1s[:], qs.pop((tS, 0))[:],
                                 start=True, stop=False)
                nc.tensor.matmul(ef[0:K, :], k2ns[:], qs.pop((tS, 1))[:],
                                 start=False, stop=True)
                aabs = softp.tile([K, L], f32, name=f"aabs{tS}", tag="aabs")
                nc.scalar.activation(aabs[:], ef[0:K, :], AF.Abs)
                expe = softp.tile([K, L], bf16, name=f"expe{tS}", tag="expe")
                nc.scalar.activation(expe[:], aabs[:], AF.Exp)
                st["expe"] = expe

            if tU >= 0:
                for j in (2, 3):
                    ups[j] = u_matmul(tU, j)

            if tS < NT:
                st = state[tS]
                sp = spp.tile([1, L], f32, name=f"sp{tS}", tag="sp")
                nc.tensor.matmul(sp[:], oncs[:], st["expe"][:],
                                 start=True, stop=True)
                rs = softp.tile([1, L], f32, name=f"rs{tS}", tag="rs")
                nc.vector.reciprocal_approx_fast(rs[:], sp[:])
                # rsb = scale/S on ACT: the 1-partition row is ~2x cheaper
                # there than the DVE tensor_scalar and GPSIMD is terrible at it
                rsb = softp.tile([1, L], bf16, name=f"rsb{tS}", tag="rsb")
                nc.scalar.activation(rsb[:], rs[:], AF.Copy,
                                     scale=scs[0:1, 0:1])
                st["rsb"] = rsb

            if tQ < NT:
                q_act(tQ, 0, q_mm(tQ, 0))

            if tU >= 0:
                u_drain_dve(tU, 2, ups[2])

            if tQ < NT:
                q_act(tQ, 1, q_mm(tQ, 1))

            if tS < NT:
                st = state[tS]
                nc.tensor.matmul(st["ef"][K:2 * K, :], onrs[:],
                                 st["rsb"][:], start=True, stop=True)
                attn = softp.tile([K, L], bf16, name=f"attn{tS}", tag="attn")
                nc.vector.tensor_mul(attn[:], st["expe"][:],
                                     st["ef"][K:2 * K, :])
                st["attn"] = attn

            if i == 0:
                v_setup()

            if tU >= 0:
                u_drain_dve(tU, 3, ups[3])
                ups[4] = u_matmul(tU, 4)
                u_drain_dve(tU, 4, ups[4])
                if tU == NT - 1:
                    # tail: stream 0 is fully drained after chunk 3 -> let
                    # its store start while stream 1 drains on DVE only
                    store_tile(tU, 0)
                for j in (5, 6, 7):
                    ups[j] = u_matmul(tU, j)
                    if tU == NT - 1:
                        u_drain_dve(tU, j, ups[j])
                    else:
                        u_drain_act_gps(tU, j, ups[j])
                if tU == NT - 1:
                    store_tile(tU, 1)
                else:
                    store_tile(tU)
                state.pop(tU)

    nc.compile()
    return nc


def _get_nc():
    if "nc" not in _CACHE:
        try:
            import concourse  # noqa: F401
        except ImportError:
            import sys
            sys.path.insert(0, "/opt/trn_rl_repo")
        _CACHE["nc"] = _build()
    return _CACHE["nc"]


def _bf16_np():
    import ml_dtypes
    return ml_dtypes.bfloat16


def _make_in_maps(inputs):
    bf = _bf16_np()

    def b16(a):
        return np.ascontiguousarray(np.asarray(a).astype(bf))

    def wpack(w_t, width):
        # [C, width] (contraction-major) -> [128, NCH*width] chunked layout
        w = np.asarray(w_t, np.float32).reshape(NCH, P, width)
        return w.transpose(1, 0, 2).reshape(P, NCH * width)

    # x: [n, c, p, t, l] for both streams -> [n, p, t, s, c, l] (tile-major)
    x1 = np.asarray(inputs["x1"], np.float32).reshape(N, NCH, P, NT, L)
    x2 = np.asarray(inputs["x2"], np.float32).reshape(N, NCH, P, NT, L)
    xall = np.stack([x1, x2], axis=1)          # [n, s, c, p, t, l]
    xds = b16(np.ascontiguousarray(xall.transpose(0, 3, 4, 1, 2, 5))
              .reshape(N, P, NT * TW))

    y1 = np.asarray(inputs["y1"])
    y2 = np.asarray(inputs["y2"])
    wq1 = wpack(np.asarray(inputs["wq1"]).T, D)
    wq2 = wpack(np.asarray(inputs["wq2"]).T, D)
    wk1 = wpack(np.asarray(inputs["wk1"]).T, D)
    wk2 = wpack(np.asarray(inputs["wk2"]).T, D)
    wv1 = wpack(np.asarray(inputs["wv1"]).T, C)
    wv2 = wpack(np.asarray(inputs["wv2"]).T, C)
    bqk = np.stack([
        np.asarray(inputs["bq1"], np.float32),
        np.asarray(inputs["bq2"], np.float32),
        np.asarray(inputs["bk1"], np.float32),
        -np.asarray(inputs["bk2"], np.float32),
    ], axis=1).astype(np.float32)              # [128, 4]
    shared = {
        "bqkd": np.ascontiguousarray(bqk),
        "bvd": b16(np.concatenate([
            np.asarray(inputs["bv1"]).reshape(-1),
            np.asarray(inputs["bv2"]).reshape(-1)]).reshape(1, 2 * C)),
        "scd": np.asarray(inputs["scale"], np.float32).reshape(1, 1),
    }
    cold = np.empty((P, CBCW), np.float32)
    for nm, (o, w) in _CBC.items():
        cold[:, o:o + w] = {"wv1": wv1, "wv2": wv2}[nm]
    cold = b16(cold)
    in_maps = []
    for i in range(N):
        m = dict(shared)
        m["xd"] = xds[i]
        blocks = {
            "y1": wpack(y1[i].T, K), "y2": wpack(y2[i].T, K),
            "wk1": wk1, "wk2": wk2, "wq1": wq1, "wq2": wq2,
        }
        hot = np.empty((P, CBHW), np.float32)
        for nm, (o, w) in _CBH.items():
            hot[:, o:o + w] = blocks[nm]
        m["cbhd"] = b16(hot)
        m["cbcd"] = cold
        in_maps.append(m)
    return in_maps


def kernel(**inputs):
    nc = _get_nc()
    from concourse.bass_utils import run_bass_kernel_spmd

    in_maps = _make_in_maps(inputs)
    res = run_bass_kernel_spmd(nc, in_maps, list(range(N))).results
    out1 = np.empty((N, C, H, W), np.float32)
    out2 = np.empty((N, C, H, W), np.float32)
    for i in range(N):
        o = np.asarray(res[i]["od"], dtype=np.float32) \
            .reshape(P, NT, 2, NCH, L)
        o = o.transpose(2, 3, 0, 1, 4)          # [s, c, p, t, l]
        out1[i] = o[0].reshape(C, H, W)
        out2[i] = o[1].reshape(C, H, W)
    return out1, out2
